# revision 36
# baseline (speedup 1.0000x reference)
"""Trainium2 Bass kernel for nn_EquivariantScalar (segment_reduce).

Network (reference.py): two gated-equivariant blocks over N=50000 atoms with
F=128 features, then a linear head and a per-molecule (B=256) masked sum.

Strategy:
- Shard atoms across the 8 cores (6250 atoms/core, padded to 6272).
- Host-side prep (part of sharding): transpose s/v to feature-major
  (f, n) layout and the one-hot mask to (n, b) layout so every DMA is
  contiguous and every matmul contracts over the partition axis; cast
  matmul operands to bf16 (mask is exactly representable).
- On-chip per 512-atom chunk: v2 = W2a@v, norm over the 3 Cartesian
  slices, mlp1+silu, mlp2 (scalar and gate halves), v1 = W1a@v gated by
  `gate`, then block 2 (v2 only - block-2's vector output is dead code for
  the scalar head), with out_w folded into block-2's mlp2 scalar half.
  Per-atom scalars are reduced per molecule by matmuls against the
  transposed mask, accumulated in PSUM across the whole kernel.
- Each core emits its (1, 256) partial; host sums the 8 partials.
"""
import sys

if "/opt/trn_rl_repo" not in sys.path:
    sys.path.insert(0, "/opt/trn_rl_repo")

import numpy as np
import ml_dtypes

import concourse.bass as bass
import concourse.mybir as mybir
import concourse.tile as tile
from concourse.tile_rust import add_dep_helper as tile_rust_add_dep
from concourse.bass_utils import run_bass_kernel_spmd

F = 128
B = 256
N_NODES = 50000
N_CORES = 8
NPC = N_NODES // N_CORES          # 6250 atoms per core
PAD = 6272                        # 49 * 128
CHUNK = 512
BF16 = mybir.dt.bfloat16
FP32 = mybir.dt.float32
AF = mybir.ActivationFunctionType
ALU = mybir.AluOpType

_CACHE = {}


def _chunks():
    out = []
    n0 = 0
    while n0 < PAD:
        w = min(CHUNK, PAD - n0)
        out.append((n0, w))
        n0 += w
    return out


def _build():
    """Build the single-core Bass program (SPMD: same program, 8 cores)."""
    nc = bass.Bass("TRN2", debug=False)

    svT = nc.dram_tensor("svT", (F, 4, PAD), BF16, kind="ExternalInput")
    mT = nc.dram_tensor("mT", (F, PAD // F, B), BF16, kind="ExternalInput")
    wnames = ["w2aT", "m1asT", "m1avT", "m2aloT", "m2ahiT", "w1aT",
              "w2bT", "m1bsT", "m1bvT"]
    wdr = {n: nc.dram_tensor(n, (F, F), BF16, kind="ExternalInput") for n in wnames}
    wdr["wcombT"] = nc.dram_tensor("wcombT", (F, 1), BF16, kind="ExternalInput")
    bnames = ["b1a", "b2alo", "b2ahi", "b1b", "bconst", "zero"]
    bdr = {n: nc.dram_tensor(n, (F, 1), FP32, kind="ExternalInput") for n in bnames}
    y_d = nc.dram_tensor("y", (1, B), FP32, kind="ExternalOutput")

    chunks = _chunks()
    nchunks = len(chunks)
    G = 7  # phase-stagger group: ACT table set switches 4x per group
    with nc.allow_low_precision(reason="bf16 intermediates are intentional"):
        with tile.TileContext(nc) as tc:
            with (
                tc.tile_pool(name="wp", bufs=1) as wp,
                tc.tile_pool(name="io", bufs=G + 3) as io,
                tc.tile_pool(name="wk", bufs=3) as wk,
                tc.tile_pool(name="ps", bufs=1, space="PSUM") as ps,
            ):
                W = {}
                for n in wnames:
                    W[n] = wp.tile([F, F], BF16, name=n + "_sb")
                nc.sync.dma_start(W["w2aT"][:], wdr["w2aT"][:])
                W["wcombT"] = wp.tile([F, 1], BF16, name="wcombT_sb")
                nc.sync.dma_start(W["wcombT"][:], wdr["wcombT"][:])
                BIA = {}
                for n in bnames:
                    BIA[n] = wp.tile([F, 1], FP32, name=n + "_sb")
                    nc.sync.dma_start(BIA[n][:], bdr[n][:])

                y_ps = ps.tile([1, B], FP32, name="y_ps", tag="y", bufs=1)

                def load_rest_of_weights():
                    for n in wnames:
                        if n != "w2aT":
                            nc.sync.dma_start(W[n][:], wdr[n][:])

                st = {}  # per-chunk live tiles
                last_act = [None]
                NB = 3  # chunks per DMA super-block
                blk = {}

                def ensure_block(bi):
                    if bi in blk or bi * NB >= nchunks:
                        return
                    cs = chunks[bi * NB:bi * NB + NB]
                    n0 = cs[0][0]
                    wt = sum(w for _, w in cs)
                    svb = io.tile([F, 4, wt], BF16, name=f"svb_{bi}", tag="lv",
                                  bufs=5)
                    if bi == 0:
                        off = 0
                        for _, w in cs:
                            nc.sync.dma_start(svb[:, :, off:off + w],
                                              svT[:, :, n0 + off:n0 + off + w])
                            off += w
                    else:
                        nc.sync.dma_start(svb[:], svT[:, :, n0:n0 + wt])
                    mb = io.tile([F, wt // F, B], BF16, name=f"mb_{bi}",
                                 tag="lm", bufs=6)
                    nc.sync.dma_start(mb[:], mT[:, n0 // F:(n0 + wt) // F, :])
                    blk[bi] = (svb, mb, n0)

                def act(*args, **kw):
                    # Chain ACT ops in emission order so the scheduler cannot
                    # interleave activation-table sets across phases.
                    inst = nc.scalar.activation(*args, **kw)
                    if last_act[0] is not None:
                        tile_rust_add_dep(inst.ins, last_act[0], sync=False,
                                          reason="act table-set ordering")
                    last_act[0] = inst.ins
                    return inst

                def a1_head(ci):
                    # Pre-issued during the previous silu phase: block loads +
                    # the first v2 matmul into the spare bank, so the next
                    # sqrt-set phase's first Square can start immediately.
                    n0, w = chunks[ci]
                    bi = ci // NB
                    ensure_block(bi)
                    svb, mb, b0 = blk[bi]
                    off = n0 - b0
                    sv_t = svb[:, :, off:off + w]
                    m_t = mb[:, off // F:(off + w) // F, :]
                    p_v2h = ps.tile([F, w], FP32, name=f"pv2h_{ci}",
                                    tag="v2h", bufs=1)
                    nc.tensor.matmul(p_v2h[:], W["w2aT"][:], sv_t[:, 1, :])
                    st[ci] = dict(s_t=sv_t[:, 0, :], v_t=sv_t[:, 1:4, :],
                                  m_t=m_t, p_v2h=p_v2h)

                def phase_a1(ci):
                    n0, w = chunks[ci]
                    if ci not in st:
                        a1_head(ci)
                    d = st[ci]
                    v_t = d["v_t"]
                    sq = wk.tile([F, 3, w], BF16, name=f"sq_{ci}", tag="sq")
                    act(sq[:, 0, :], d.pop("p_v2h")[:], AF.Square,
                        bias=BIA["zero"][:])
                    for c in (1, 2):
                        p_v2 = ps.tile([F, w], FP32, name=f"pv2_{ci}_{c}",
                                       tag="pa", bufs=4)
                        nc.tensor.matmul(p_v2[:], W["w2aT"][:], v_t[:, c, :])
                        act(sq[:, c, :], p_v2[:], AF.Square, bias=BIA["zero"][:])
                    t01 = wk.tile([F, w], BF16, name=f"t01_{ci}", tag="t01")
                    nc.vector.tensor_tensor(t01[:], sq[:, 0, :], sq[:, 1, :],
                                            ALU.add)
                    nsq = wk.tile([F, w], BF16, name=f"nsq_{ci}", tag="nsq")
                    nc.vector.tensor_tensor(nsq[:], t01[:], sq[:, 2, :], ALU.add)
                    v2n = wk.tile([F, w], BF16, name=f"v2n_{ci}", tag="v2n",
                                  bufs=G + 1)
                    act(v2n[:], nsq[:], AF.Sqrt, bias=BIA["zero"][:])
                    d["v2n"] = v2n

                def b1_mlp1(ci):
                    n0, w = chunks[ci]
                    d = st[ci]
                    p_h1 = ps.tile([F, w], FP32, name=f"ph1_{ci}", tag="h1",
                                   bufs=2)
                    nc.tensor.matmul(p_h1[:], W["m1asT"][:], d["s_t"],
                                     start=True, stop=False)
                    nc.tensor.matmul(p_h1[:], W["m1avT"][:], d["v2n"][:],
                                     start=False, stop=True)
                    d["p_h1"] = p_h1

                def phase_b1(ci):
                    n0, w = chunks[ci]
                    d = st[ci]
                    if "p_h1" not in d:
                        b1_mlp1(ci)
                    p_h1 = d.pop("p_h1")
                    h1 = wk.tile([F, w], BF16, name=f"h1_{ci}", tag="h1s")
                    act(h1[:], p_h1[:], AF.Silu, bias=BIA["b1a"][:])
                    p_h2lo = ps.tile([F, w], FP32, name=f"ph2lo_{ci}",
                                     tag="pa", bufs=4)
                    nc.tensor.matmul(p_h2lo[:], W["m2aloT"][:], h1[:])
                    s1 = wk.tile([F, w], BF16, name=f"s1_{ci}", tag="s1",
                                 bufs=G + 2)
                    act(s1[:], p_h2lo[:], AF.Identity, bias=BIA["b2alo"][:])
                    p_h2hi = ps.tile([F, w], FP32, name=f"ph2hi_{ci}",
                                     tag="pa", bufs=4)
                    nc.tensor.matmul(p_h2hi[:], W["m2ahiT"][:], h1[:])
                    gate = wk.tile([F, w], BF16, name=f"gate_{ci}", tag="gate")
                    act(gate[:], p_h2hi[:], AF.Identity, bias=BIA["b2ahi"][:])
                    vo = wk.tile([F, 3, w], BF16, name=f"vo_{ci}", tag="vo",
                                 bufs=G + 2)
                    for c in range(3):
                        p_v1 = ps.tile([F, w], FP32, name=f"pv1_{ci}_{c}",
                                       tag="pa", bufs=4)
                        nc.tensor.matmul(p_v1[:], W["w1aT"][:], d["v_t"][:, c, :])
                        nc.vector.tensor_tensor(vo[:, c, :], p_v1[:], gate[:],
                                                ALU.mult)
                    d["vo"] = vo
                    d["s1"] = s1

                def phase_a2(ci):
                    n0, w = chunks[ci]
                    d = st[ci]
                    sqb = wk.tile([F, 3, w], BF16, name=f"sqb_{ci}", tag="sq")
                    cpb = wk.tile([F, 2, w], BF16, name=f"cpb_{ci}", tag="cpb")
                    for c in range(3):
                        p_v2b = ps.tile([F, w], FP32, name=f"pv2b_{ci}_{c}",
                                        tag="pa", bufs=4)
                        nc.tensor.matmul(p_v2b[:], W["w2bT"][:],
                                         d["vo"][:, c, :])
                        if c == 0:
                            act(sqb[:, c, :], p_v2b[:], AF.Square,
                                bias=BIA["zero"][:])
                        else:
                            nc.vector.tensor_copy(cpb[:, c - 1, :], p_v2b[:])
                            nc.vector.tensor_tensor(sqb[:, c, :],
                                                    cpb[:, c - 1, :],
                                                    cpb[:, c - 1, :], ALU.mult)
                    t01b = wk.tile([F, w], BF16, name=f"t01b_{ci}", tag="t01")
                    nc.vector.tensor_tensor(t01b[:], sqb[:, 0, :], sqb[:, 1, :],
                                            ALU.add)
                    nsqb = wk.tile([F, w], BF16, name=f"nsqb_{ci}", tag="nsq")
                    nc.vector.tensor_tensor(nsqb[:], t01b[:], sqb[:, 2, :],
                                            ALU.add)
                    v2nb = wk.tile([F, w], BF16, name=f"v2nb_{ci}", tag="v2nb",
                                   bufs=G + 2)
                    act(v2nb[:], nsqb[:], AF.Sqrt, bias=BIA["zero"][:])
                    d["v2nb"] = v2nb

                def phase_b2(ci):
                    n0, w = chunks[ci]
                    d = st[ci]
                    p_hb = ps.tile([F, w], FP32, name=f"phb_{ci}", tag="h1",
                                   bufs=2)
                    nc.tensor.matmul(p_hb[:], W["m1bsT"][:], d["s1"][:],
                                     start=True, stop=False)
                    nc.tensor.matmul(p_hb[:], W["m1bvT"][:], d["v2nb"][:],
                                     start=False, stop=True)
                    hb = wk.tile([F, w], BF16, name=f"hb_{ci}", tag="hb",
                                 bufs=G + 2)
                    act(hb[:], p_hb[:], AF.Silu, bias=BIA["b1b"][:])
                    d["hb"] = hb

                def b2_tail(ci):
                    # sf + y matmuls: deferred into the next sqrt-set phase
                    # to densify the PE there (no silu-set ACT ops needed)
                    n0, w = chunks[ci]
                    nsub = w // F
                    d = st[ci]
                    hb = d["hb"]
                    p_sf = ps.tile([F, nsub], FP32, name=f"psf_{ci}", tag="pa",
                                   bufs=4)
                    for i in range(nsub):
                        nc.tensor.matmul(p_sf[:, i:i + 1],
                                         hb[:, i * F:(i + 1) * F],
                                         W["wcombT"][:])
                    sf = wk.tile([F, nsub], BF16, name=f"sf_{ci}", tag="sf")
                    nc.vector.tensor_scalar_add(sf[:], p_sf[:], BIA["bconst"][:])
                    for i in range(nsub):
                        first = ci == 0 and i == 0
                        last = ci == nchunks - 1 and i == nsub - 1
                        nc.tensor.matmul(y_ps[:], sf[:, i:i + 1],
                                         d["m_t"][:, i, :],
                                         start=first, stop=last,
                                         skip_group_check=True)
                    del st[ci]

                sizes = [3, 5, 5]
                groups = []
                c0 = 0
                for s in sizes:
                    groups.append(list(range(c0, min(c0 + s, nchunks))))
                    c0 += s
                for gi, grp in enumerate(groups):
                    prev = groups[gi - 1] if gi > 0 else []
                    prev2 = groups[gi - 2] if gi > 1 else []
                    # sqrt-set phase: this group's norm-1 + previous group's
                    # block-2 norms + older tails; pre-issue the first
                    # silu-phase mlp1 matmuls mid-phase
                    for k in range(max(len(grp), len(prev), len(prev2))):
                        if k < len(prev):
                            phase_a2(prev[k])
                        if k < len(grp):
                            phase_a1(grp[k])
                        if k == 1:
                            if gi == 0:
                                load_rest_of_weights()
                                ensure_block(1)
                            b1_mlp1(grp[0])
                        if k == 2 and len(grp) > 1:
                            b1_mlp1(grp[1])
                        if k < len(prev2):
                            b2_tail(prev2[k])
                    # silu-set phase: this group's B1 (incl. block-2 matmuls
                    # and squares) + previous group's B2 tail
                    for k in range(max(len(grp), len(prev))):
                        if k < len(grp):
                            phase_b1(grp[k])
                        if k == 1 and gi + 1 < len(groups):
                            a1_head(groups[gi + 1][0])
                        if k == 3 and gi + 1 < len(groups):
                            ensure_block(groups[gi + 1][0] // NB + 1)
                        if k < len(prev):
                            phase_b2(prev[k])
                last = groups[-1]
                prelast = groups[-2] if len(groups) > 1 else []
                for cj in last:
                    phase_a2(cj)
                for cj in prelast:
                    b2_tail(cj)
                for cj in last:
                    phase_b2(cj)
                for cj in last:
                    b2_tail(cj)

                y_sb = wk.tile([1, B], FP32, name="y_sb", tag="ysb")
                nc.vector.tensor_copy(y_sb[:], y_ps[:])
                nc.sync.dma_start(y_d[:], y_sb[:])

    _dedupe_ldweights(nc)
    # Walrus in this container only accepts one sync-wait per instruction;
    # split multi-wait instructions into NoOp chains.
    _split_sync_waits_inline(nc, max_waits=1)
    return nc


def _dedupe_ldweights(nc):
    """Drop LDWEIGHTS whose weight AP is identical to the previous load on
    the PE stream (the stationary operand is still resident). Sync waits of
    removed loads are transplanted onto the next PE instruction."""
    f = nc.m.functions[0]
    removed = 0
    for blk in f.blocks:
        new_insts = []
        last_sig = None
        pending_waits = []
        for inst in blk.instructions:
            tn = type(inst).__name__
            if getattr(inst, "engine", None) != mybir.EngineType.PE:
                new_insts.append(inst)
                continue
            if tn == "InstLdweights":
                ap = inst.ins[0]
                sig = (ap.memref, ap.offset, str(ap.ap), str(ap.dtype),
                       str(getattr(inst, "perf_mode", None)))
                if sig == last_sig:
                    si = inst.sync_info
                    if si is not None:
                        pending_waits.extend(si.on_wait or [])
                        assert not si.on_update
                    removed += 1
                    continue
                last_sig = sig
            elif tn == "InstMatmult":
                if getattr(inst, "is_transpose", False):
                    last_sig = None
            if pending_waits:
                si = inst.sync_info
                old_w = list(si.on_wait) if si and si.on_wait else []
                old_u = list(si.on_update) if si and si.on_update else []
                inst.sync_info = mybir.SyncInfo(
                    on_wait=pending_waits + old_w, on_update=old_u)
                pending_waits = []
            new_insts.append(inst)
        assert not pending_waits
        blk.instructions[:] = new_insts
    return removed


def _split_sync_waits_inline(nc, max_waits=1):
    f = nc.m.functions[0]
    counter = [0]
    for blk in f.blocks:
        new_insts = []
        for inst in blk.instructions:
            si = getattr(inst, "sync_info", None)
            waits = list(si.on_wait) if si and si.on_wait else []
            if len(waits) > max_waits:
                head, rest = waits[:-max_waits], waits[-max_waits:]
                for i in range(0, len(head), max_waits):
                    counter[0] += 1
                    nop = mybir.InstNoOp(
                        name=f"I-wsplit-{counter[0]}",
                        engine=inst.engine,
                        ins=[],
                        outs=[],
                        sync_info=mybir.SyncInfo(
                            on_wait=head[i:i + max_waits], on_update=[]),
                    )
                    new_insts.append(nop)
                inst.sync_info = mybir.SyncInfo(on_wait=rest,
                                                on_update=list(si.on_update))
            new_insts.append(inst)
        blk.instructions[:] = new_insts


def _get_nc():
    if "nc" not in _CACHE:
        _CACHE["nc"] = _build()
    return _CACHE["nc"]


def _prep_inputs(s, v, batch_mask, w1, w2, mlp_w1, mlp_b1, mlp_w2, mlp_b2,
                 out_w, out_b):
    bf16 = ml_dtypes.bfloat16
    s = np.asarray(s, np.float32)
    v = np.asarray(v, np.float32)
    batch_mask = np.asarray(batch_mask, np.float32)

    shared = {}

    def wt(name, arr):
        shared[name] = np.ascontiguousarray(arr.astype(bf16))

    wt("w2aT", np.asarray(w2)[0].T)
    wt("w1aT", np.asarray(w1)[0].T)
    wt("w2bT", np.asarray(w2)[1].T)
    m1a = np.asarray(mlp_w1)[0]
    wt("m1asT", m1a[:, :F].T)
    wt("m1avT", m1a[:, F:].T)
    m2a = np.asarray(mlp_w2)[0]
    wt("m2aloT", m2a[:F, :].T)
    wt("m2ahiT", m2a[F:, :].T)
    m1b = np.asarray(mlp_w1)[1]
    wt("m1bsT", m1b[:, :F].T)
    wt("m1bvT", m1b[:, F:].T)
    out_w = np.asarray(out_w, np.float32)
    out_b = np.asarray(out_b, np.float32)
    m2b = np.asarray(mlp_w2)[1]
    wcomb = out_w[0] @ m2b[:F, :]                      # (F,)
    wt("wcombT", wcomb[:, None])
    bconst = float(out_w[0] @ np.asarray(mlp_b2)[1][:F] + out_b[0])

    b1 = np.asarray(mlp_b1, np.float32)
    b2 = np.asarray(mlp_b2, np.float32)
    shared["b1a"] = np.ascontiguousarray(b1[0][:, None], dtype=np.float32)
    shared["b2alo"] = np.ascontiguousarray(b2[0][:F][:, None], dtype=np.float32)
    shared["b2ahi"] = np.ascontiguousarray(b2[0][F:][:, None], dtype=np.float32)
    shared["b1b"] = np.ascontiguousarray(b1[1][:, None], dtype=np.float32)
    shared["bconst"] = np.full((F, 1), bconst, dtype=np.float32)
    shared["zero"] = np.zeros((F, 1), dtype=np.float32)

    in_maps = []
    for k in range(N_CORES):
        lo, hi = k * NPC, (k + 1) * NPC
        sk = np.zeros((PAD, F), np.float32)
        sk[:NPC] = s[0, lo:hi]
        vk = np.zeros((PAD, 3, F), np.float32)
        vk[:NPC] = v[0, lo:hi]
        mk = np.zeros((PAD, B), np.float32)
        mk[:NPC] = batch_mask[:, lo:hi, 0].T
        m = dict(shared)
        sv = np.empty((F, 4, PAD), np.float32)
        sv[:, 0, :] = sk.T
        sv[:, 1:4, :] = vk.transpose(2, 1, 0)
        m["svT"] = np.ascontiguousarray(sv.astype(bf16))
        m["mT"] = np.ascontiguousarray(
            mk.reshape(PAD // F, F, B).transpose(1, 0, 2).astype(bf16))
        in_maps.append(m)
    return in_maps


def run(inputs, trace=False, **kw):
    nc = _get_nc()
    in_maps = _prep_inputs(
        inputs["s"], inputs["v"], inputs["batch_mask"], inputs["w1"],
        inputs["w2"], inputs["mlp_w1"], inputs["mlp_b1"], inputs["mlp_w2"],
        inputs["mlp_b2"], inputs["out_w"], inputs["out_b"])
    res = run_bass_kernel_spmd(nc, in_maps, list(range(N_CORES)),
                               trace=trace, **kw)
    y = np.zeros((1, B), np.float64)
    for k in range(N_CORES):
        y += res.results[k]["y"].astype(np.float64)
    return y.astype(np.float32).reshape(B, 1), res


def kernel(**inputs):
    y, _ = run(inputs)
    return y


# revision 37
# speedup vs baseline: 1.0510x; 1.0510x over previous
"""Trainium2 Bass kernel for nn_EquivariantScalar (segment_reduce).

Network (reference.py): two gated-equivariant blocks over N=50000 atoms with
F=128 features, then a linear head and a per-molecule (B=256) masked sum.

Strategy:
- Shard atoms across the 8 cores (6250 atoms/core, padded to 6272).
- Host-side prep (part of sharding): transpose s/v to feature-major
  (f, n) layout and the one-hot mask to (n, b) layout so every DMA is
  contiguous and every matmul contracts over the partition axis; cast
  matmul operands to bf16 (mask is exactly representable).
- On-chip per 512-atom chunk: v2 = W2a@v, norm over the 3 Cartesian
  slices, mlp1+silu, mlp2 (scalar and gate halves), v1 = W1a@v gated by
  `gate`, then block 2 (v2 only - block-2's vector output is dead code for
  the scalar head), with out_w folded into block-2's mlp2 scalar half.
  Per-atom scalars are reduced per molecule by matmuls against the
  transposed mask, accumulated in PSUM across the whole kernel.
- Each core emits its (1, 256) partial; host sums the 8 partials.
"""
import sys

if "/opt/trn_rl_repo" not in sys.path:
    sys.path.insert(0, "/opt/trn_rl_repo")

import numpy as np
import ml_dtypes

import concourse.bass as bass
import concourse.mybir as mybir
import concourse.tile as tile
from concourse.tile_rust import add_dep_helper as tile_rust_add_dep
from concourse.bass_utils import run_bass_kernel_spmd

F = 128
B = 256
N_NODES = 50000
N_CORES = 8
NPC = N_NODES // N_CORES          # 6250 atoms per core
PAD = 6272                        # 49 * 128
CHUNK = 512
BF16 = mybir.dt.bfloat16
FP32 = mybir.dt.float32
AF = mybir.ActivationFunctionType
ALU = mybir.AluOpType

_CACHE = {}


def _chunks():
    out = []
    n0 = 0
    while n0 < PAD:
        w = min(CHUNK, PAD - n0)
        out.append((n0, w))
        n0 += w
    return out


def _build():
    """Build the single-core Bass program (SPMD: same program, 8 cores)."""
    nc = bass.Bass("TRN2", debug=False)

    svT = nc.dram_tensor("svT", (F, 4, PAD), BF16, kind="ExternalInput")
    mT = nc.dram_tensor("mT", (F, PAD // F, B), BF16, kind="ExternalInput")
    wnames = ["w2aT", "m1asT", "m1avT", "m2aloT", "m2ahiT", "w1aT",
              "w2bT", "m1bsT", "m1bvT"]
    wdr = {n: nc.dram_tensor(n, (F, F), BF16, kind="ExternalInput") for n in wnames}
    wdr["wcombT"] = nc.dram_tensor("wcombT", (F, 1), BF16, kind="ExternalInput")
    bnames = ["b1a", "b2alo", "b2ahi", "b1b", "bconst", "zero"]
    bdr = {n: nc.dram_tensor(n, (F, 1), FP32, kind="ExternalInput") for n in bnames}
    y_d = nc.dram_tensor("y", (1, B), FP32, kind="ExternalOutput")

    chunks = _chunks()
    nchunks = len(chunks)
    G = 7  # phase-stagger group: ACT table set switches 4x per group
    with nc.allow_low_precision(reason="bf16 intermediates are intentional"):
        with tile.TileContext(nc) as tc:
            with (
                tc.tile_pool(name="wp", bufs=1) as wp,
                tc.tile_pool(name="io", bufs=G + 3) as io,
                tc.tile_pool(name="wk", bufs=3) as wk,
                tc.tile_pool(name="ps", bufs=1, space="PSUM") as ps,
            ):
                W = {}
                for n in wnames:
                    W[n] = wp.tile([F, F], BF16, name=n + "_sb")
                nc.sync.dma_start(W["w2aT"][:], wdr["w2aT"][:])
                W["wcombT"] = wp.tile([F, 1], BF16, name="wcombT_sb")
                nc.sync.dma_start(W["wcombT"][:], wdr["wcombT"][:])
                BIA = {}
                for n in bnames:
                    BIA[n] = wp.tile([F, 1], FP32, name=n + "_sb")
                    nc.sync.dma_start(BIA[n][:], bdr[n][:])

                y_ps = ps.tile([1, B], FP32, name="y_ps", tag="y", bufs=1)

                def load_rest_of_weights():
                    for n in wnames:
                        if n != "w2aT":
                            nc.sync.dma_start(W[n][:], wdr[n][:])

                st = {}  # per-chunk live tiles
                last_act = [None]
                NB = 3  # chunks per DMA super-block
                blk = {}

                def ensure_block(bi):
                    if bi in blk or bi * NB >= nchunks:
                        return
                    cs = chunks[bi * NB:bi * NB + NB]
                    n0 = cs[0][0]
                    wt = sum(w for _, w in cs)
                    svb = io.tile([F, 4, wt], BF16, name=f"svb_{bi}", tag="lv",
                                  bufs=5)
                    if bi == 0:
                        off = 0
                        for _, w in cs:
                            nc.sync.dma_start(svb[:, :, off:off + w],
                                              svT[:, :, n0 + off:n0 + off + w])
                            off += w
                    else:
                        nc.sync.dma_start(svb[:], svT[:, :, n0:n0 + wt])
                    mb = io.tile([F, wt // F, B], BF16, name=f"mb_{bi}",
                                 tag="lm", bufs=6)
                    nc.sync.dma_start(mb[:], mT[:, n0 // F:(n0 + wt) // F, :])
                    blk[bi] = (svb, mb, n0)

                def act(*args, **kw):
                    # Chain ACT ops in emission order so the scheduler cannot
                    # interleave activation-table sets across phases.
                    inst = nc.scalar.activation(*args, **kw)
                    if last_act[0] is not None:
                        tile_rust_add_dep(inst.ins, last_act[0], sync=False,
                                          reason="act table-set ordering")
                    last_act[0] = inst.ins
                    return inst

                def a1_head(ci):
                    # Pre-issued during the previous silu phase: block loads +
                    # the first v2 matmul into the spare bank, so the next
                    # sqrt-set phase's first Square can start immediately.
                    n0, w = chunks[ci]
                    bi = ci // NB
                    ensure_block(bi)
                    svb, mb, b0 = blk[bi]
                    off = n0 - b0
                    sv_t = svb[:, :, off:off + w]
                    m_t = mb[:, off // F:(off + w) // F, :]
                    p_v2h = ps.tile([F, w], FP32, name=f"pv2h_{ci}",
                                    tag="v2h", bufs=1)
                    nc.tensor.matmul(p_v2h[:], W["w2aT"][:], sv_t[:, 1, :])
                    st[ci] = dict(s_t=sv_t[:, 0, :], v_t=sv_t[:, 1:4, :],
                                  m_t=m_t, p_v2h=p_v2h)

                def phase_a1(ci):
                    n0, w = chunks[ci]
                    if ci not in st:
                        a1_head(ci)
                    d = st[ci]
                    v_t = d["v_t"]
                    sq = wk.tile([F, 3, w], BF16, name=f"sq_{ci}", tag="sq")
                    act(sq[:, 0, :], d.pop("p_v2h")[:], AF.Square,
                        bias=BIA["zero"][:])
                    for c in (1, 2):
                        p_v2 = ps.tile([F, w], FP32, name=f"pv2_{ci}_{c}",
                                       tag="pa", bufs=4)
                        nc.tensor.matmul(p_v2[:], W["w2aT"][:], v_t[:, c, :])
                        act(sq[:, c, :], p_v2[:], AF.Square, bias=BIA["zero"][:])
                    t01 = wk.tile([F, w], BF16, name=f"t01_{ci}", tag="t01")
                    nc.vector.tensor_tensor(t01[:], sq[:, 0, :], sq[:, 1, :],
                                            ALU.add)
                    nsq = wk.tile([F, w], BF16, name=f"nsq_{ci}", tag="nsq")
                    nc.vector.tensor_tensor(nsq[:], t01[:], sq[:, 2, :], ALU.add)
                    v2n = wk.tile([F, w], BF16, name=f"v2n_{ci}", tag="v2n",
                                  bufs=G + 1)
                    act(v2n[:], nsq[:], AF.Sqrt, bias=BIA["zero"][:])
                    d["v2n"] = v2n

                def b1_mlp1(ci):
                    n0, w = chunks[ci]
                    d = st[ci]
                    p_h1 = ps.tile([F, w], FP32, name=f"ph1_{ci}", tag="h1",
                                   bufs=2)
                    nc.tensor.matmul(p_h1[:], W["m1asT"][:], d["s_t"],
                                     start=True, stop=False)
                    nc.tensor.matmul(p_h1[:], W["m1avT"][:], d["v2n"][:],
                                     start=False, stop=True)
                    d["p_h1"] = p_h1

                def phase_b1(ci):
                    n0, w = chunks[ci]
                    d = st[ci]
                    if "p_h1" not in d:
                        b1_mlp1(ci)
                    p_h1 = d.pop("p_h1")
                    h1 = wk.tile([F, w], BF16, name=f"h1_{ci}", tag="h1s")
                    act(h1[:], p_h1[:], AF.Silu, bias=BIA["b1a"][:])
                    p_h2lo = ps.tile([F, w], FP32, name=f"ph2lo_{ci}",
                                     tag="pa", bufs=4)
                    nc.tensor.matmul(p_h2lo[:], W["m2aloT"][:], h1[:])
                    s1 = wk.tile([F, w], BF16, name=f"s1_{ci}", tag="s1",
                                 bufs=G + 2)
                    act(s1[:], p_h2lo[:], AF.Identity, bias=BIA["b2alo"][:])
                    p_h2hi = ps.tile([F, w], FP32, name=f"ph2hi_{ci}",
                                     tag="pa", bufs=4)
                    nc.tensor.matmul(p_h2hi[:], W["m2ahiT"][:], h1[:])
                    gate = wk.tile([F, w], BF16, name=f"gate_{ci}", tag="gate")
                    act(gate[:], p_h2hi[:], AF.Identity, bias=BIA["b2ahi"][:])
                    vo = wk.tile([F, 3, w], BF16, name=f"vo_{ci}", tag="vo",
                                 bufs=G + 2)
                    for c in range(3):
                        p_v1 = ps.tile([F, w], FP32, name=f"pv1_{ci}_{c}",
                                       tag="pa", bufs=4)
                        nc.tensor.matmul(p_v1[:], W["w1aT"][:], d["v_t"][:, c, :])
                        nc.vector.tensor_tensor(vo[:, c, :], p_v1[:], gate[:],
                                                ALU.mult)
                    d["vo"] = vo
                    d["s1"] = s1

                def phase_a2(ci):
                    n0, w = chunks[ci]
                    d = st[ci]
                    sqb = wk.tile([F, 3, w], BF16, name=f"sqb_{ci}", tag="sq")
                    cpb = wk.tile([F, 2, w], BF16, name=f"cpb_{ci}", tag="cpb")
                    for c in range(3):
                        p_v2b = ps.tile([F, w], FP32, name=f"pv2b_{ci}_{c}",
                                        tag="pa", bufs=4)
                        nc.tensor.matmul(p_v2b[:], W["w2bT"][:],
                                         d["vo"][:, c, :])
                        if c == 0:
                            act(sqb[:, c, :], p_v2b[:], AF.Square,
                                bias=BIA["zero"][:])
                        else:
                            nc.vector.tensor_copy(cpb[:, c - 1, :], p_v2b[:])
                            nc.vector.tensor_tensor(sqb[:, c, :],
                                                    cpb[:, c - 1, :],
                                                    cpb[:, c - 1, :], ALU.mult)
                    t01b = wk.tile([F, w], BF16, name=f"t01b_{ci}", tag="t01")
                    nc.vector.tensor_tensor(t01b[:], sqb[:, 0, :], sqb[:, 1, :],
                                            ALU.add)
                    nsqb = wk.tile([F, w], BF16, name=f"nsqb_{ci}", tag="nsq")
                    nc.vector.tensor_tensor(nsqb[:], t01b[:], sqb[:, 2, :],
                                            ALU.add)
                    v2nb = wk.tile([F, w], BF16, name=f"v2nb_{ci}", tag="v2nb",
                                   bufs=G + 2)
                    act(v2nb[:], nsqb[:], AF.Sqrt, bias=BIA["zero"][:])
                    d["v2nb"] = v2nb

                def phase_b2(ci):
                    n0, w = chunks[ci]
                    d = st[ci]
                    p_hb = ps.tile([F, w], FP32, name=f"phb_{ci}", tag="h1",
                                   bufs=2)
                    nc.tensor.matmul(p_hb[:], W["m1bsT"][:], d["s1"][:],
                                     start=True, stop=False)
                    nc.tensor.matmul(p_hb[:], W["m1bvT"][:], d["v2nb"][:],
                                     start=False, stop=True)
                    hb = wk.tile([F, w], BF16, name=f"hb_{ci}", tag="hb",
                                 bufs=G + 2)
                    act(hb[:], p_hb[:], AF.Silu, bias=BIA["b1b"][:])
                    d["hb"] = hb

                def b2_tail(ci):
                    # sf + y matmuls: deferred into the next sqrt-set phase
                    # to densify the PE there (no silu-set ACT ops needed)
                    n0, w = chunks[ci]
                    nsub = w // F
                    d = st[ci]
                    hb = d["hb"]
                    p_sf = ps.tile([F, nsub], FP32, name=f"psf_{ci}", tag="pa",
                                   bufs=4)
                    for i in range(nsub):
                        nc.tensor.matmul(p_sf[:, i:i + 1],
                                         hb[:, i * F:(i + 1) * F],
                                         W["wcombT"][:])
                    sf = wk.tile([F, nsub], BF16, name=f"sf_{ci}", tag="sf")
                    nc.vector.tensor_scalar_add(sf[:], p_sf[:], BIA["bconst"][:])
                    for i in range(nsub):
                        first = ci == 0 and i == 0
                        last = ci == nchunks - 1 and i == nsub - 1
                        nc.tensor.matmul(y_ps[:], sf[:, i:i + 1],
                                         d["m_t"][:, i, :],
                                         start=first, stop=last,
                                         skip_group_check=True)
                    del st[ci]

                sizes = [3, 5, 5]
                groups = []
                c0 = 0
                for s in sizes:
                    groups.append(list(range(c0, min(c0 + s, nchunks))))
                    c0 += s
                for gi, grp in enumerate(groups):
                    prev = groups[gi - 1] if gi > 0 else []
                    prev2 = groups[gi - 2] if gi > 1 else []
                    # sqrt-set phase: this group's norm-1 + previous group's
                    # block-2 norms + older tails; pre-issue the first
                    # silu-phase mlp1 matmuls mid-phase
                    for k in range(max(len(grp), len(prev), len(prev2))):
                        if k < len(grp):
                            phase_a1(grp[k])
                        if k == 1:
                            if gi == 0:
                                load_rest_of_weights()
                                ensure_block(1)
                            b1_mlp1(grp[0])
                        if k == 2 and len(grp) > 1:
                            b1_mlp1(grp[1])
                        if k < len(prev2):
                            b2_tail(prev2[k])
                        if k < len(prev):
                            phase_a2(prev[k])
                    # silu-set phase: this group's B1 (incl. block-2 matmuls
                    # and squares) + previous group's B2 tail
                    for k in range(max(len(grp), len(prev))):
                        if k < len(grp):
                            phase_b1(grp[k])
                        if k == 1 and gi + 1 < len(groups):
                            a1_head(groups[gi + 1][0])
                        if k == 3 and gi + 1 < len(groups):
                            ensure_block(groups[gi + 1][0] // NB + 1)
                        if k < len(prev):
                            phase_b2(prev[k])
                last = groups[-1]
                prelast = groups[-2] if len(groups) > 1 else []
                for cj in last:
                    phase_a2(cj)
                for cj in prelast:
                    b2_tail(cj)
                for cj in last:
                    phase_b2(cj)
                for cj in last:
                    b2_tail(cj)

                y_sb = wk.tile([1, B], FP32, name="y_sb", tag="ysb")
                nc.vector.tensor_copy(y_sb[:], y_ps[:])
                nc.sync.dma_start(y_d[:], y_sb[:])

    _dedupe_ldweights(nc)
    # Walrus in this container only accepts one sync-wait per instruction;
    # split multi-wait instructions into NoOp chains.
    _split_sync_waits_inline(nc, max_waits=1)
    return nc


def _dedupe_ldweights(nc):
    """Drop LDWEIGHTS whose weight AP is identical to the previous load on
    the PE stream (the stationary operand is still resident). Sync waits of
    removed loads are transplanted onto the next PE instruction."""
    f = nc.m.functions[0]
    removed = 0
    for blk in f.blocks:
        new_insts = []
        last_sig = None
        pending_waits = []
        for inst in blk.instructions:
            tn = type(inst).__name__
            if getattr(inst, "engine", None) != mybir.EngineType.PE:
                new_insts.append(inst)
                continue
            if tn == "InstLdweights":
                ap = inst.ins[0]
                sig = (ap.memref, ap.offset, str(ap.ap), str(ap.dtype),
                       str(getattr(inst, "perf_mode", None)))
                if sig == last_sig:
                    si = inst.sync_info
                    if si is not None:
                        pending_waits.extend(si.on_wait or [])
                        assert not si.on_update
                    removed += 1
                    continue
                last_sig = sig
            elif tn == "InstMatmult":
                if getattr(inst, "is_transpose", False):
                    last_sig = None
            if pending_waits:
                si = inst.sync_info
                old_w = list(si.on_wait) if si and si.on_wait else []
                old_u = list(si.on_update) if si and si.on_update else []
                inst.sync_info = mybir.SyncInfo(
                    on_wait=pending_waits + old_w, on_update=old_u)
                pending_waits = []
            new_insts.append(inst)
        assert not pending_waits
        blk.instructions[:] = new_insts
    return removed


def _split_sync_waits_inline(nc, max_waits=1):
    f = nc.m.functions[0]
    counter = [0]
    for blk in f.blocks:
        new_insts = []
        for inst in blk.instructions:
            si = getattr(inst, "sync_info", None)
            waits = list(si.on_wait) if si and si.on_wait else []
            if len(waits) > max_waits:
                head, rest = waits[:-max_waits], waits[-max_waits:]
                for i in range(0, len(head), max_waits):
                    counter[0] += 1
                    nop = mybir.InstNoOp(
                        name=f"I-wsplit-{counter[0]}",
                        engine=inst.engine,
                        ins=[],
                        outs=[],
                        sync_info=mybir.SyncInfo(
                            on_wait=head[i:i + max_waits], on_update=[]),
                    )
                    new_insts.append(nop)
                inst.sync_info = mybir.SyncInfo(on_wait=rest,
                                                on_update=list(si.on_update))
            new_insts.append(inst)
        blk.instructions[:] = new_insts


def _get_nc():
    if "nc" not in _CACHE:
        _CACHE["nc"] = _build()
    return _CACHE["nc"]


def _prep_inputs(s, v, batch_mask, w1, w2, mlp_w1, mlp_b1, mlp_w2, mlp_b2,
                 out_w, out_b):
    bf16 = ml_dtypes.bfloat16
    s = np.asarray(s, np.float32)
    v = np.asarray(v, np.float32)
    batch_mask = np.asarray(batch_mask, np.float32)

    shared = {}

    def wt(name, arr):
        shared[name] = np.ascontiguousarray(arr.astype(bf16))

    wt("w2aT", np.asarray(w2)[0].T)
    wt("w1aT", np.asarray(w1)[0].T)
    wt("w2bT", np.asarray(w2)[1].T)
    m1a = np.asarray(mlp_w1)[0]
    wt("m1asT", m1a[:, :F].T)
    wt("m1avT", m1a[:, F:].T)
    m2a = np.asarray(mlp_w2)[0]
    wt("m2aloT", m2a[:F, :].T)
    wt("m2ahiT", m2a[F:, :].T)
    m1b = np.asarray(mlp_w1)[1]
    wt("m1bsT", m1b[:, :F].T)
    wt("m1bvT", m1b[:, F:].T)
    out_w = np.asarray(out_w, np.float32)
    out_b = np.asarray(out_b, np.float32)
    m2b = np.asarray(mlp_w2)[1]
    wcomb = out_w[0] @ m2b[:F, :]                      # (F,)
    wt("wcombT", wcomb[:, None])
    bconst = float(out_w[0] @ np.asarray(mlp_b2)[1][:F] + out_b[0])

    b1 = np.asarray(mlp_b1, np.float32)
    b2 = np.asarray(mlp_b2, np.float32)
    shared["b1a"] = np.ascontiguousarray(b1[0][:, None], dtype=np.float32)
    shared["b2alo"] = np.ascontiguousarray(b2[0][:F][:, None], dtype=np.float32)
    shared["b2ahi"] = np.ascontiguousarray(b2[0][F:][:, None], dtype=np.float32)
    shared["b1b"] = np.ascontiguousarray(b1[1][:, None], dtype=np.float32)
    shared["bconst"] = np.full((F, 1), bconst, dtype=np.float32)
    shared["zero"] = np.zeros((F, 1), dtype=np.float32)

    in_maps = []
    for k in range(N_CORES):
        lo, hi = k * NPC, (k + 1) * NPC
        sk = np.zeros((PAD, F), np.float32)
        sk[:NPC] = s[0, lo:hi]
        vk = np.zeros((PAD, 3, F), np.float32)
        vk[:NPC] = v[0, lo:hi]
        mk = np.zeros((PAD, B), np.float32)
        mk[:NPC] = batch_mask[:, lo:hi, 0].T
        m = dict(shared)
        sv = np.empty((F, 4, PAD), np.float32)
        sv[:, 0, :] = sk.T
        sv[:, 1:4, :] = vk.transpose(2, 1, 0)
        m["svT"] = np.ascontiguousarray(sv.astype(bf16))
        m["mT"] = np.ascontiguousarray(
            mk.reshape(PAD // F, F, B).transpose(1, 0, 2).astype(bf16))
        in_maps.append(m)
    return in_maps


def run(inputs, trace=False, **kw):
    nc = _get_nc()
    in_maps = _prep_inputs(
        inputs["s"], inputs["v"], inputs["batch_mask"], inputs["w1"],
        inputs["w2"], inputs["mlp_w1"], inputs["mlp_b1"], inputs["mlp_w2"],
        inputs["mlp_b2"], inputs["out_w"], inputs["out_b"])
    res = run_bass_kernel_spmd(nc, in_maps, list(range(N_CORES)),
                               trace=trace, **kw)
    y = np.zeros((1, B), np.float64)
    for k in range(N_CORES):
        y += res.results[k]["y"].astype(np.float64)
    return y.astype(np.float32).reshape(B, 1), res


def kernel(**inputs):
    y, _ = run(inputs)
    return y
